# revision 43
# baseline (speedup 1.0000x reference)
"""Trainium2 Bass kernel for nn_DSCBR (gnn_message_passing).

Strategy (8 NeuronCores, SPMD, dest-row sharding):
- Each graph's nodes are split into per-side regions (U/I resp. U/B),
  round-robin sharded; node tables stored bf16 padded to 256B rows so
  SWDGE dma_gather (elem_size=128 bf16) output feeds TensorE directly.
- SpMM: bank-major loop. A PSUM bank [128,512] holds 4 dest win-pairs;
  per 128-edge chunk one merged selection matrix (single is_equal vs a
  256-wide iota) drives 2 matmuls (win-pair halves) accumulating in
  PSUM across all source windows; one fused drain per bank does
  norm-epilogue + acc add + bf16 staging for the next-layer table.
- Gathers round-robin 4 SWDGE queues (descriptor-gen parallelism).
- AllGathers between layers (padded bf16) and for loss tables
  (unpadded bf16) overlap the next phase's gathers.
- Losses computed batch-sharded (256 rows/core) + tiny AllReduce.
"""
import os
import sys
import types

sys.path.insert(0, "/opt/trn_rl_repo")

import numpy as np

import concourse.bass as bass
import concourse.bacc as bacc
import concourse.mybir as mybir
import concourse.tile as tile
from concourse.bass_utils import run_bass_kernel_spmd
from concourse.masks import make_identity

P = 128
NCORES = 8
SRC_WIN = 32768
GI_CH = 20                    # chunks per gather batch (<= 2560 idx)
D = 64
NU, NI, NB = 100000, 50000, 20000
BATCH = 2048
F32 = mybir.dt.float32
BF = mybir.dt.bfloat16
I32 = mybir.dt.int32
I16 = mybir.dt.int16
AF = mybir.ActivationFunctionType
ALU = mybir.AluOpType

# per-core padded slot counts (multiples of 256)
PCU = 12544                   # 100000/8 = 12500 -> 49 wp
PCI = 6400                    # 50000/8  = 6250  -> 25 wp
PCB = 2560                    # 20000/8  = 2500  -> 10 wp
VU, VI, VB = PCU * NCORES, PCI * NCORES, PCB * NCORES   # 100352, 51200, 20480
R1 = PCU + PCI                # 18944, wps 0..73 (U 0..48, I 49..73)
R2 = PCU + PCB                # 15104, wps 0..58 (U 0..48, B 49..58)
NW1, NW2, NWB = R1 // P, R2 // P, PCB // P


def mapU(x):
    return (x % NCORES) * PCU + x // NCORES


def mapI(x):
    return (x % NCORES) * PCI + x // NCORES


def mapB(x):
    return (x % NCORES) * PCB + x // NCORES


# ---------------------------------------------------------------- host prep

def build_graph_side(dst_core, dst_wp, dst_lrow, src_idx, vals, wp0, nwp_side,
                     v_src, first_win):
    """One side of a graph: per-(core,wp,window) chunk counts (SPMD-max over
    cores) and the sorted per-core edge segments.

    Returns (banks, seg) where banks = [(bank_wp0, bank_nwp,
    [(win, [chunks...]), ...])] with chunks = (wpi, start, stop), and seg
    maps (core, wp, win) -> (slice into the sorted arrays)."""
    nwin = (v_src + SRC_WIN - 1) // SRC_WIN
    win = src_idx // SRC_WIN
    wloc = (src_idx % SRC_WIN).astype(np.int16)

    counts = np.zeros((NCORES, nwp_side, nwin), np.int64)
    np.add.at(counts, (dst_core, dst_wp - wp0, win), 1)
    nch = (counts.max(axis=0) + P - 1) // P          # [nwp_side, nwin]
    for w in range(nwp_side):
        if nch[w].sum() == 0:
            nch[w, 0] = 1

    order = np.lexsort((dst_lrow, win, dst_wp, dst_core))
    s_wp = dst_wp[order] - wp0
    s_win = win[order]
    s_core = dst_core[order]
    s_wloc = wloc[order]
    s_lrow = dst_lrow[order]
    s_val = vals[order]

    key = (s_core * nwp_side + s_wp) * nwin + s_win
    starts = np.searchsorted(key, np.arange(NCORES * nwp_side * nwin))
    ends = np.searchsorted(key, np.arange(NCORES * nwp_side * nwin) + 1)

    banks = []
    for b0 in range(0, nwp_side, 4):
        bnwp = min(4, nwp_side - b0)
        wins = []
        for w in range(nwin):
            chunks = []
            for wpi in range(bnwp):
                wp = b0 + wpi
                c = int(nch[wp, w])
                if c == 0:
                    continue
                ws = np.nonzero(nch[wp])[0]
                st_w, sp_w = ws[0], ws[-1]
                for k in range(c):
                    chunks.append((wpi, w == st_w and k == 0,
                                   w == sp_w and k == c - 1))
            if chunks:
                wins.append((w, chunks))
        banks.append((b0 + wp0, bnwp, wins))
    info = dict(banks=banks, nch=nch, starts=starts, ends=ends,
                s_wloc=s_wloc, s_lrow=s_lrow, s_val=s_val,
                nwin=nwin, wp0=wp0, nwp=nwp_side, first_win=first_win)
    return info


def fill_side_streams(info, idx_s, lr_s, val_s, pos0):
    """Append this side's padded streams for all cores in emit order.
    Returns new stream position (in edges)."""
    nwin, wp0, nwp = info['nwin'], info['wp0'], info['nwp']
    nch, starts, ends = info['nch'], info['starts'], info['ends']
    pos_out = pos0
    for c in range(NCORES):
        pos = pos0
        for (b0, bnwp, wins) in info['banks']:
            for (w, chunks) in wins:
                for wp in range(b0, b0 + bnwp):
                    ncw = int(nch[wp - wp0, w])
                    if ncw == 0:
                        continue
                    k = (c * nwp + (wp - wp0)) * nwin + w
                    a, b = starts[k], ends[k]
                    n = b - a
                    idx_s[c, pos:pos + n] = info['s_wloc'][a:b]
                    lr_s[c, pos:pos + n] = info['s_lrow'][a:b]
                    val_s[c, pos:pos + n] = info['s_val'][a:b]
                    pos += ncw * P
        pos_out = pos
    return pos_out


def wrap_idx16(flat):
    # index i -> partition i%16, col i//16; replicated x8 down partitions
    return np.ascontiguousarray(np.tile(flat.reshape(-1, 16).T.astype(np.int16), (8, 1)))


def stream_cols(a, dtype):
    # [ncores, n] -> [ncores, 128, n/128] column-chunk layout
    n = a.shape[1]
    return np.ascontiguousarray(
        a.reshape(NCORES, -1, P).transpose(0, 2, 1)).astype(dtype)


def idx_cols_i32(flat):
    n = flat.shape[0]
    assert n % P == 0
    return np.ascontiguousarray(flat.reshape(-1, P).T.astype(np.int32))


def build_graph(rows, cols, vals, kind):
    """kind: 'il' (U|I combined ids), 'bl' (U|B), 'ag' (bundle rows, item cols).
    Returns dict(sides=[sideinfo...], streams=(idx, lr, val), tot)."""
    rows = np.asarray(rows).astype(np.int64)
    cols = np.asarray(cols).astype(np.int64)
    vals = np.asarray(vals, np.float32)
    if kind == "il":
        n_first, map_dst2, side_v = NU, mapI, (VI, VU)
        nwp_a, nwp_b = PCU // 256, PCI // 256
    elif kind == "bl":
        n_first, map_dst2, side_v = NU, mapB, (VB, VU)
        nwp_a, nwp_b = PCU // 256, PCB // 256
    else:  # ag
        d = mapB(rows)
        core, slot = d // PCB, d % PCB
        s = build_graph_side(core, slot // 256, (slot % 256).astype(np.float32),
                             mapI(cols), vals, 0, PCB // 256, VI, 0)
        s['src'] = 'I1acc'
        s['v_src'] = VI
        tot = int(sum(c for (_, _, wins) in s['banks']
                      for (_, ch) in wins for c in [len(ch)])) * P
        idx_s = np.zeros((NCORES, tot), np.int16)
        lr_s = np.full((NCORES, tot), 300.0, np.float32)
        val_s = np.zeros((NCORES, tot), np.float32)
        fill_side_streams(s, idx_s, lr_s, val_s, 0)
        return dict(sides=[s], idx=idx_s, lr=lr_s, val=val_s, tot=tot)

    first = rows < n_first                      # dst in U region
    # side A: dst U, src = second region
    dA = mapU(rows[first])
    srcA = map_dst2(cols[first] - n_first)
    coreA, slotA = dA // PCU, dA % PCU
    sA = build_graph_side(coreA, slotA // 256, (slotA % 256).astype(np.float32),
                          srcA, vals[first], 0, nwp_a, side_v[0], 0)
    # side B: dst second region, src U
    d2 = map_dst2(rows[~first] - n_first)
    src2 = mapU(cols[~first])
    pc2 = PCI if kind == "il" else PCB
    core2, slot2 = d2 // pc2, d2 % pc2
    sB = build_graph_side(core2, nwp_a + slot2 // 256,
                          (slot2 % 256).astype(np.float32),
                          src2, vals[~first], nwp_a, nwp_b, side_v[1], 0)
    sA['src'] = 'I1' if kind == 'il' else 'B2'
    sA['v_src'] = side_v[0]
    sB['src'] = 'U1' if kind == 'il' else 'U2'
    sB['v_src'] = side_v[1]
    tot = 0
    for s in (sA, sB):
        tot += int(sum(len(ch) for (_, _, wins) in s['banks']
                       for (_, ch) in wins)) * P
    idx_s = np.zeros((NCORES, tot), np.int16)
    lr_s = np.full((NCORES, tot), 300.0, np.float32)
    val_s = np.zeros((NCORES, tot), np.float32)
    p = fill_side_streams(sA, idx_s, lr_s, val_s, 0)
    fill_side_streams(sB, idx_s, lr_s, val_s, p)
    return dict(sides=[sA, sB], idx=idx_s, lr=lr_s, val=val_s, tot=tot)


def preprocess(inputs):
    import ml_dtypes
    u = np.asarray(inputs["users_feature"], np.float32)
    it = np.asarray(inputs["items_feature"], np.float32)
    b = np.asarray(inputs["bundles_feature"], np.float32)

    def padded_table(feat, mapper, v):
        t = np.zeros((v, 128), ml_dtypes.bfloat16)
        t[mapper(np.arange(feat.shape[0])), :D] = feat.astype(ml_dtypes.bfloat16)
        return t

    f0_U = padded_table(u, mapU, VU)
    f0_I = padded_table(it, mapI, VI)
    f0_B = padded_table(b, mapB, VB)

    # per-core f32 shards for acc init (region layout [U | second])
    f0_il_sh = np.zeros((NCORES, R1, D), np.float32)
    f0_bl_sh = np.zeros((NCORES, R2, D), np.float32)
    for c in range(NCORES):
        nu_c = len(range(c, NU, NCORES))
        f0_il_sh[c, :nu_c] = u[c::NCORES]
        f0_bl_sh[c, :nu_c] = u[c::NCORES]
        ni_c = len(range(c, NI, NCORES))
        f0_il_sh[c, PCU:PCU + ni_c] = it[c::NCORES]
        nb_c = len(range(c, NB, NCORES))
        f0_bl_sh[c, PCU:PCU + nb_c] = b[c::NCORES]

    il = build_graph(inputs["il_row"], inputs["il_col"],
                     np.asarray(inputs["il_val"], np.float32), "il")
    bl = build_graph(inputs["bl_row"], inputs["bl_col"],
                     np.asarray(inputs["bl_val"], np.float32), "bl")
    ag = build_graph(np.asarray(inputs["agg_row"]), np.asarray(inputs["agg_col"]),
                     np.asarray(inputs["agg_val"], np.float32), "ag")

    users = np.asarray(inputs["users"]).astype(np.int64)
    bundles = np.asarray(inputs["bundles"]).astype(np.int64)
    bsh = BATCH // NCORES
    loss = {}
    for c in range(NCORES):
        sl = slice(c * bsh, (c + 1) * bsh)
        loss[c] = dict(
            u_il=idx_cols_i32(mapU(users[sl])),
            b_il0=idx_cols_i32(mapB(bundles[sl, 0])),
            b_il1=idx_cols_i32(mapB(bundles[sl, 1])),
        )
    aug_u = idx_cols_i32(mapU(users))
    aug_b0 = idx_cols_i32(mapB(bundles[:, 0]))

    return dict(f0_U=f0_U, f0_I=f0_I, f0_B=f0_B,
                f0_il_sh=f0_il_sh, f0_bl_sh=f0_bl_sh,
                il=il, bl=bl, ag=ag, loss=loss,
                aug_u=aug_u, aug_b0=aug_b0)


# ---------------------------------------------------------------- bass build

class Ctx:
    pass


def emit_graph(cx, g, tables, meta, acc_sb, f1_map, scale, pos0, name,
               hook=None):
    """One SpMM layer over graph g (host program). f1_map: side-src-key ->
    (dram tile, padded:bool) for next-layer table write, or None (layer 2).
    acc_sb: [128, nw*64] f32 accumulator (norm epilogue) or None (AG copy).
    hook: called once after the first batch (to emit collectives/exports of
    the PREVIOUS phase without blocking this phase's gather dispatch)."""
    nc = cx.nc
    lr_sb, val_sb = meta
    pos = pos0
    pending_drain = [None]
    hook_pending = [hook]
    for s in g['sides']:
        table_ap = tables[s['src']]
        v_src = s['v_src']
        for (wp0, bnwp, wins) in s['banks']:
            S = 2 * bnwp
            Sl = S * D
            # one accumulation group per PSUM bank: start=True clears the
            # has_written bits of the WHOLE bank, so only the bank's first
            # matmul may set it (per-element first-touch then overwrites,
            # later touches accumulate).
            nmm = 2 * sum(len(ch) for (_, ch) in wins)
            mm = 0
            psum = cx.psp.tile([P, 512], F32, space="PSUM", tag="pbank",
                               name="pbank", bufs=4)
            for (w, chunks) in wins:
                lo = w * SRC_WIN
                rows = min(SRC_WIN, v_src - lo)
                src_slice = table_ap[lo:lo + rows, :]
                for c0 in range(0, len(chunks), GI_CH):
                    batch = chunks[c0:c0 + GI_CH]
                    nch = len(batch)
                    gi = nch * P
                    idx_t = cx.idxp.tile([128, GI_CH * 8], I16, tag="gidx",
                                         name="gidx", bufs=9)
                    nc.sync.dma_start(
                        out=idx_t[:, :gi // 16],
                        in_=cx.g_idx[name[:2]][:, pos // 16:pos // 16 + gi // 16])
                    gt = cx.gp.tile([P, GI_CH * 128], BF, tag="gg", name="gg",
                                    bufs=8)
                    nc.gpsimd.dma_gather(
                        out_ap=gt[:, :nch * 128].rearrange(
                            "p (c d) -> p c d", c=nch),
                        in_ap=src_slice,
                        idxs_ap=idx_t[:, :gi // 16],
                        num_idxs=gi,
                        num_idxs_reg=gi,
                        elem_size=128,
                        single_packet=False,
                        queue_num=cx.qrr % 4,
                    )
                    cx.qrr += 1
                    gv = cx.gvp.tile([P, GI_CH * D], BF, tag="gv", name="gv",
                                     bufs=6)
                    nc.vector.tensor_mul(
                        gv[:, :nch * D].rearrange("p (c d) -> p c d", c=nch),
                        gt[:, :nch * 128].rearrange(
                            "p (c d) -> p c d", c=nch)[:, :, 0:D],
                        val_sb[:, pos // P:pos // P + nch].to_broadcast(
                            [P, nch, D]))
                    sel = cx.selp.tile([P, GI_CH * 256], BF, tag="sel",
                                       name="sel", bufs=3)
                    nc.vector.tensor_tensor(
                        out=sel[:, :nch * 256].rearrange(
                            "p (c j) -> p c j", c=nch),
                        in0=cx.iota_rep[:, :nch * 256].rearrange(
                            "p (c j) -> p c j", c=nch),
                        in1=lr_sb[:, pos // P:pos // P + nch].to_broadcast(
                            [P, nch, 256]),
                        op=ALU.is_equal)
                    for k, (wpi, st, sp) in enumerate(batch):
                        nc.tensor.matmul(
                            out=psum[:, (2 * wpi) * D:(2 * wpi + 1) * D],
                            lhsT=sel[:, k * 256:k * 256 + 128],
                            rhs=gv[:, k * D:(k + 1) * D],
                            start=(mm == 0), stop=False)
                        nc.tensor.matmul(
                            out=psum[:, (2 * wpi + 1) * D:(2 * wpi + 2) * D],
                            lhsT=sel[:, k * 256 + 128:k * 256 + 256],
                            rhs=gv[:, k * D:(k + 1) * D],
                            start=False, stop=(mm == nmm - 2))
                        mm += 2
                    pos += gi
                    if hook_pending[0] is not None:
                        hook_pending[0]()
                        hook_pending[0] = None
                    # drain of the previous bank, deferred until this bank's
                    # first batch is queued (keeps it off the vector queue
                    # head while the previous bank's matmuls finish)
                    if pending_drain[0] is not None:
                        pending_drain[0]()
                        pending_drain[0] = None
            pending_drain[0] = _make_drain(cx, s, wp0, S, Sl, psum, acc_sb,
                                           f1_map, scale)
    pending_drain[0]()
    return pos


def _make_drain(cx, s, wp0, S, Sl, psum, acc_sb, f1_map, scale):
    nc = cx.nc

    def drain():
            w0 = wp0 * 2
            if acc_sb is not None:
                praw = cx.dp.tile([P, 512], F32, tag="d_pr", name="d_pr",
                                  bufs=2)
                nc.scalar.activation(praw[:, :Sl], psum[:, :Sl], AF.Copy)
                sq = cx.dp.tile([P, 512], F32, tag="d_sq", name="d_sq", bufs=2)
                nc.scalar.activation(sq[:, :Sl], praw[:, :Sl], AF.Square)
                ss = cx.dp.tile([P, 8], F32, tag="d_ss", name="d_ss", bufs=2)
                nc.vector.reduce_sum(
                    ss[:, :S], sq[:, :Sl].rearrange("p (s d) -> p s d", s=S),
                    axis=mybir.AxisListType.X)
                sn = cx.dp.tile([P, 8], F32, tag="d_sn", name="d_sn", bufs=2)
                nc.scalar.activation(sn[:, :S], ss[:, :S], AF.Sqrt)
                nc.vector.tensor_scalar_max(sn[:, :S], sn[:, :S], 1e-12)
                rn = cx.dp.tile([P, 8], F32, tag="d_rn", name="d_rn", bufs=2)
                nc.vector.reciprocal(rn[:, :S], sn[:, :S])
                contrib = cx.dp.tile([P, 512], F32, tag="d_ct", name="d_ct",
                                     bufs=2)
                nc.vector.tensor_mul(
                    contrib[:, :Sl].rearrange("p (s d) -> p s d", s=S),
                    praw[:, :Sl].rearrange("p (s d) -> p s d", s=S),
                    rn[:, :S].to_broadcast([P, S, D]))
                nc.vector.tensor_add(acc_sb[:, w0 * D:w0 * D + Sl],
                                     acc_sb[:, w0 * D:w0 * D + Sl],
                                     contrib[:, :Sl])
                stg_src = praw
            else:
                stg_src = psum
            if f1_map is not None:
                f1_t, padded = f1_map[s['src2']]
                stg = cx.stp.tile([P, 8 * D], BF, tag="d_st", name="d_st",
                                  bufs=2)
                nc.scalar.activation(stg[:, :Sl], stg_src[:, :Sl], AF.Copy,
                                     scale=scale)
                r0 = (wp0 - s['wp0']) * 256
                if padded:
                    dst = f1_t[r0:r0 + S * P, 0:D]
                else:
                    dst = f1_t[r0:r0 + S * P, :]
                nc.scalar.dma_start(
                    out=dst.rearrange("(s p) d -> p s d", p=P),
                    in_=stg[:, :Sl].rearrange("p (s d) -> p s d", s=S))

    return drain


def indirect_gather_rows(cx, out_sb, table_ap, idx_sb, ncols):
    nc = cx.nc
    for k in range(ncols):
        nc.gpsimd.indirect_dma_start(
            out=out_sb[:, k * D:(k + 1) * D],
            out_offset=None,
            in_=table_ap,
            in_offset=bass.IndirectOffsetOnAxis(ap=idx_sb[:, k:k + 1], axis=0),
        )


def normalize_rows(cx, x_sb, ngroups, tag):
    nc = cx.nc
    sq = cx.lp.tile([P, ngroups * D], F32, tag=f"{tag}_sq")
    nc.vector.tensor_mul(sq[:], x_sb[:, :ngroups * D], x_sb[:, :ngroups * D])
    ss = cx.lp.tile([P, ngroups], F32, tag=f"{tag}_ss")
    nc.vector.reduce_sum(ss[:], sq[:].rearrange("p (w d) -> p w d", w=ngroups),
                         axis=mybir.AxisListType.X)
    sn = cx.lp.tile([P, ngroups], F32, tag=f"{tag}_sn")
    nc.scalar.activation(sn[:], ss[:], AF.Sqrt)
    nc.vector.tensor_scalar_max(sn[:], sn[:], 1e-12)
    rn = cx.lp.tile([P, ngroups], F32, tag=f"{tag}_rn")
    nc.vector.reciprocal(rn[:], sn[:])
    nc.vector.tensor_mul(
        x_sb[:, :ngroups * D].rearrange("p (w d) -> p w d", w=ngroups),
        x_sb[:, :ngroups * D].rearrange("p (w d) -> p w d", w=ngroups),
        rn[:].to_broadcast([P, ngroups, D]),
    )


def rowdot(cx, a_sb, b_sb, out_sb, ngroups, tag):
    nc = cx.nc
    t = cx.lp.tile([P, ngroups * D], F32, tag=f"{tag}_t")
    nc.vector.tensor_mul(t[:], a_sb[:, :ngroups * D], b_sb[:, :ngroups * D])
    nc.vector.reduce_sum(out_sb[:, :ngroups],
                         t[:].rearrange("p (w d) -> p w d", w=ngroups),
                         axis=mybir.AxisListType.X)


def transpose_groups(cx, src_sb, ngroups, tag):
    nc = cx.nc
    out = cx.lp.tile([P, ngroups * P], F32, tag=f"{tag}_T")
    for gi in range(ngroups):
        pt = cx.psp.tile([P, P], F32, space="PSUM", tag="tr_ps", bufs=1)
        nc.tensor.transpose(out=pt[:D, :P], in_=src_sb[:, gi * D:(gi + 1) * D],
                            identity=cx.ident[:])
        nc.vector.tensor_copy(out[:D, gi * P:(gi + 1) * P], pt[:D, :P])
    return out


def build(pp, debug_tables=False):
    nc = bacc.Bacc("TRN2", target_bir_lowering=False, debug=False,
                   num_devices=NCORES, num_swdge_queues=4)
    cx = Ctx()
    cx.nc = nc
    cx.qrr = 0

    # ---- dram inputs
    f0_U = nc.dram_tensor("f0_U", [VU, 128], BF, kind="ExternalInput")
    f0_I = nc.dram_tensor("f0_I", [VI, 128], BF, kind="ExternalInput")
    f0_B = nc.dram_tensor("f0_B", [VB, 128], BF, kind="ExternalInput")
    f0_il_sh = nc.dram_tensor("f0_il_sh", [R1, D], F32, kind="ExternalInput")
    f0_bl_sh = nc.dram_tensor("f0_bl_sh", [R2, D], F32, kind="ExternalInput")
    cx.g_idx = {}
    g_meta = {}
    for gname in ("il", "bl", "ag"):
        tot = pp[gname]['tot']
        cx.g_idx[gname] = nc.dram_tensor(f"{gname}_idx", [128, tot // 16], I16,
                                         kind="ExternalInput")
        g_meta[gname] = (
            nc.dram_tensor(f"{gname}_lr", [128, tot // P], BF,
                           kind="ExternalInput"),
            nc.dram_tensor(f"{gname}_val", [128, tot // P], BF,
                           kind="ExternalInput"))
    lidx = {k: nc.dram_tensor(f"loss_{k}", [128, 2], I32, kind="ExternalInput")
            for k in ("u_il", "b_il0", "b_il1")}
    lidx["aug_u"] = nc.dram_tensor("loss_aug_u", [128, 16], I32,
                                   kind="ExternalInput")
    lidx["aug_b0"] = nc.dram_tensor("loss_aug_b0", [128, 16], I32,
                                    kind="ExternalInput")
    out_t = nc.dram_tensor("out", [1, 2], F32, kind="ExternalOutput")
    dbg = {}

    # side src2 keys (which f1 tile a bank's dst side writes)
    for g, keys in (("il", ("U1f", "I1f")), ("bl", ("U2f", "B2f")),
                    ("ag", ("ilb",))):
        for s, k in zip(pp[g]['sides'], keys):
            s['src2'] = k

    with tile.TileContext(nc) as tc:
        cx.tc = tc
        es = []

        def pool(name, bufs, **kw):
            p = tc.tile_pool(name=name, bufs=bufs, **kw)
            es.append(p)
            return p.__enter__()

        cx.psp = pool("psum", 2, space="PSUM")
        cx.dramp = pool("dram", 1, space="DRAM")
        cx.cp = pool("const", 1)

        iota_i = cx.cp.tile([P, 256], I32)
        nc.gpsimd.iota(iota_i[:], pattern=[[1, 256]], base=0,
                       channel_multiplier=0)
        cx.iota_bf = cx.cp.tile([P, 256], BF)
        nc.vector.tensor_copy(cx.iota_bf[:], iota_i[:])
        cx.iota_rep = cx.cp.tile([P, GI_CH * 256], BF)
        nc.vector.tensor_copy(
            cx.iota_rep[:].rearrange("p (c j) -> p c j", c=GI_CH),
            cx.iota_bf[:].rearrange("p (o j) -> p o j", o=1).to_broadcast(
                [P, GI_CH, 256]))
        cx.ident = cx.cp.tile([P, P], F32)
        make_identity(nc, cx.ident[:])
        ones_col = cx.cp.tile([P, 1], F32)
        nc.vector.memset(ones_col[:], 1.0)

        def ag_pair(nm, rows_in, rows_out, cols, dtype=BF):
            ain = cx.dramp.tile([rows_in, cols], dtype, tag=f"{nm}_i",
                                name=f"{nm}_i")
            aout = cx.dramp.tile([rows_out, cols], dtype, addr_space="Shared",
                                 tag=f"{nm}_o", name=f"{nm}_o")
            return ain, aout

        def allgather(pair):
            nc.gpsimd.collective_compute(
                "AllGather", ALU.bypass, replica_groups=[list(range(NCORES))],
                ins=[pair[0][:].opt()], outs=[pair[1][:].opt()])

        # ---------- SpMM phases in a scoped pool block ----------
        es2 = []

        def pool2(name, bufs, **kw):
            p = tc.tile_pool(name=name, bufs=bufs, **kw)
            es2.append(p)
            return p.__enter__()

        cx.gp = pool2("gather", 4)
        cx.idxp = pool2("gidx", 6)
        cx.gvp = pool2("gval", 3)
        cx.selp = pool2("sel", 3)
        cx.dp = pool2("drain", 2)
        cx.stp = pool2("stage", 2)
        cx.mp = pool2("meta", 1)
        cx.accp = pool2("accs", 1)
        cx.xp = pool2("export", 1)

        meta_sb = {}
        for gname in ("il", "bl", "ag"):
            tot = pp[gname]['tot']
            lr = cx.mp.tile([128, tot // P], BF, tag=f"{gname}_lr",
                            name=f"{gname}_lr")
            vv = cx.mp.tile([128, tot // P], BF, tag=f"{gname}_vv",
                            name=f"{gname}_vv")
            nc.scalar.dma_start(out=lr[:], in_=g_meta[gname][0][:])
            nc.scalar.dma_start(out=vv[:], in_=g_meta[gname][1][:])
            meta_sb[gname] = (lr, vv)

        acc_il = cx.accp.tile([P, NW1 * D], F32, tag="acc_il", name="acc_il")
        nc.scalar.dma_start(out=acc_il[:].rearrange("p (w d) -> p w d", w=NW1),
                            in_=f0_il_sh[:].rearrange("(w p) d -> p w d", p=P))
        acc_bl = cx.accp.tile([P, NW2 * D], F32, tag="acc_bl", name="acc_bl")
        nc.scalar.dma_start(out=acc_bl[:].rearrange("p (w d) -> p w d", w=NW2),
                            in_=f0_bl_sh[:].rearrange("(w p) d -> p w d", p=P))

        il_f1_U = ag_pair("ilf1U", PCU, VU, 128)
        il_f1_I = ag_pair("ilf1I", PCI, VI, 128)
        bl_f1_U = ag_pair("blf1U", PCU, VU, 128)
        bl_f1_B = ag_pair("blf1B", PCB, VB, 128)
        acc_il_I = ag_pair("accilI", PCI, VI, 128)
        acc_il_U = ag_pair("accilU", PCU, VU, D)
        acc_bl_U = ag_pair("accblU", PCU, VU, D)
        acc_bl_B = ag_pair("accblB", PCB, VB, D)
        ilb = ag_pair("ilb", PCB, VB, D)

        def export(acc_t, col0, nwin, dst, pad):
            stg = cx.xp.tile([P, (PCU // P) * D], BF, tag="xstg", name="xstg",
                             bufs=1)
            nc.scalar.activation(stg[:, :nwin * D],
                                 acc_t[:, col0 * D:(col0 + nwin) * D], AF.Copy)
            d = dst[:, 0:D] if pad else dst[:]
            nc.scalar.dma_start(
                out=d.rearrange("(s p) d -> p s d", p=P),
                in_=stg[:, :nwin * D].rearrange("p (s d) -> p s d", s=nwin))

        # ---- IL layer 1
        emit_graph(cx, pp['il'], {'I1': f0_I[:], 'U1': f0_U[:]},
                   meta_sb['il'], acc_il,
                   {'U1f': (il_f1_U[0], True), 'I1f': (il_f1_I[0], True)},
                   0.5, 0, "il1")
        # ---- BL layer 1 (il_f1 allgathers dispatched behind its first batch)
        emit_graph(cx, pp['bl'], {'B2': f0_B[:], 'U2': f0_U[:]},
                   meta_sb['bl'], acc_bl,
                   {'U2f': (bl_f1_U[0], True), 'B2f': (bl_f1_B[0], True)},
                   0.5, 0, "bl1",
                   hook=lambda: (allgather(il_f1_U), allgather(il_f1_I)))
        # ---- IL layer 2
        emit_graph(cx, pp['il'], {'I1': il_f1_I[1][:], 'U1': il_f1_U[1][:]},
                   meta_sb['il'], acc_il, None, 1.0, 0, "il2",
                   hook=lambda: (allgather(bl_f1_U), allgather(bl_f1_B)))

        # ---- BL layer 2 (acc_il exports + allgathers behind first batch)
        def hook_bl2():
            export(acc_il, PCU // P, PCI // P, acc_il_I[0], True)
            export(acc_il, 0, PCU // P, acc_il_U[0], False)
            allgather(acc_il_I)
            allgather(acc_il_U)
        emit_graph(cx, pp['bl'], {'B2': bl_f1_B[1][:], 'U2': bl_f1_U[1][:]},
                   meta_sb['bl'], acc_bl, None, 1.0, 0, "bl2", hook=hook_bl2)

        # ---- AG spmm (copy drains into ilb)
        def hook_ag():
            export(acc_bl, 0, PCU // P, acc_bl_U[0], False)
            export(acc_bl, PCU // P, PCB // P, acc_bl_B[0], False)
            allgather(acc_bl_U)
            allgather(acc_bl_B)
        emit_graph(cx, pp['ag'], {'I1acc': acc_il_I[1][:]},
                   meta_sb['ag'], None, {'ilb': (ilb[0], False)},
                   1.0, 0, "ag", hook=hook_ag)
        allgather(ilb)

        for p in reversed(es2):
            p.__exit__(None, None, None)
        cx.lp = pool("loss", 1)

        if debug_tables:
            for nm, t, rows, cols in (
                    ("dbg_acc_il_U", acc_il_U[1], VU, D),
                    ("dbg_acc_il_I", acc_il_I[1], VI, 128),
                    ("dbg_acc_bl_U", acc_bl_U[1], VU, D),
                    ("dbg_acc_bl_B", acc_bl_B[1], VB, D),
                    ("dbg_ilb", ilb[1], VB, D),
                    ("dbg_f1_U", il_f1_U[1], VU, 128),
                    ("dbg_f1_I", il_f1_I[1], VI, 128)):
                o = nc.dram_tensor(nm, [rows, cols], BF, kind="ExternalOutput")
                nc.sync.dma_start(out=o[:], in_=t[:])
                dbg[nm] = o

        # ---------------- loss phase ----------------
        bsh = BATCH // NCORES
        ng = bsh // P
        lidx_sb = {}
        for k, t in lidx.items():
            s = cx.lp.tile([128, t.shape[1]], I32, tag=f"li_{k}")
            nc.sync.dma_start(out=s[:], in_=t[:])
            lidx_sb[k] = s

        def gather(tag, table, idxk, ncols):
            sb_bf = cx.lp.tile([P, ncols * D], BF, tag=f"{tag}_bf")
            indirect_gather_rows(cx, sb_bf, table, lidx_sb[idxk], ncols)
            sb = cx.lp.tile([P, ncols * D], F32, tag=tag)
            nc.scalar.activation(sb[:], sb_bf[:], AF.Copy)
            return sb

        pos_u_il = gather("pos_u_il", acc_il_U[1][:], "u_il", ng)
        pos_u_bl = gather("pos_u_bl", acc_bl_U[1][:], "u_il", ng)
        b_il0 = gather("b_il0", ilb[1][:], "b_il0", ng)
        b_il1 = gather("b_il1", ilb[1][:], "b_il1", ng)
        b_bl0 = gather("b_bl0", acc_bl_B[1][:], "b_il0", ng)
        b_bl1 = gather("b_bl1", acc_bl_B[1][:], "b_il1", ng)
        aug_u = gather("aug_u", acc_bl_U[1][:], "aug_u", 16)
        aug_b = gather("aug_b", acc_bl_B[1][:], "aug_b0", 16)
        # -- bpr
        pr0 = cx.lp.tile([P, ng], F32, tag="pr0")
        pr1 = cx.lp.tile([P, ng], F32, tag="pr1")
        tmp = cx.lp.tile([P, ng], F32, tag="prt")
        rowdot(cx, pos_u_il, b_il0, pr0, ng, "d0")
        rowdot(cx, pos_u_bl, b_bl0, tmp, ng, "d1")
        nc.vector.tensor_add(pr0[:], pr0[:], tmp[:])
        rowdot(cx, pos_u_il, b_il1, pr1, ng, "d2")
        rowdot(cx, pos_u_bl, b_bl1, tmp, ng, "d3")
        nc.vector.tensor_add(pr1[:], pr1[:], tmp[:])
        x = cx.lp.tile([P, ng], F32, tag="bprx")
        nc.vector.tensor_tensor(out=x[:], in0=pr1[:], in1=pr0[:],
                                op=ALU.subtract)
        negx = cx.lp.tile([P, ng], F32, tag="bprnx")
        nc.vector.tensor_scalar_mul(negx[:], x[:], -1.0)
        nax = cx.lp.tile([P, ng], F32, tag="bprax")
        nc.vector.tensor_tensor(out=nax[:], in0=x[:], in1=negx[:], op=ALU.min)
        e = cx.lp.tile([P, ng], F32, tag="bpre")
        nc.scalar.activation(e[:], nax[:], AF.Exp)
        nc.vector.tensor_scalar_add(e[:], e[:], 1.0)
        l1p = cx.lp.tile([P, ng], F32, tag="bprl")
        nc.scalar.activation(l1p[:], e[:], AF.Ln)
        sp = cx.lp.tile([P, ng], F32, tag="bprsp")
        nc.vector.tensor_scalar_max(sp[:], x[:], 0.0)
        nc.vector.tensor_add(sp[:], sp[:], l1p[:])

        # -- contrastive partials
        aug_u_my = cx.lp.tile([P, ng * D], F32, tag="aug_u_my")
        nc.vector.tensor_copy(aug_u_my[:], pos_u_bl[:, :ng * D])
        aug_b_my = cx.lp.tile([P, ng * D], F32, tag="aug_b_my")
        nc.vector.tensor_copy(aug_b_my[:], b_bl0[:, :ng * D])
        my_pos_b = cx.lp.tile([P, ng * D], F32, tag="my_pb")
        nc.vector.tensor_copy(my_pos_b[:], b_il0[:, :ng * D])

        normalize_rows(cx, aug_u, 16, "nau")
        normalize_rows(cx, aug_b, 16, "nab")
        normalize_rows(cx, pos_u_il, ng, "npu")
        normalize_rows(cx, my_pos_b, ng, "npb")
        normalize_rows(cx, aug_u_my, ng, "naum")
        normalize_rows(cx, aug_b_my, ng, "nabm")

        part = cx.lp.tile([P, 4], F32, tag="parts")
        nc.vector.memset(part[:], 0.0)
        nc.vector.reduce_sum(part[:, 0:1],
                             sp[:].rearrange("p (w d) -> p w d", w=1),
                             axis=mybir.AxisListType.X)

        def closs_partial(pos_my, aug_full, aug_my_cols, out_col):
            posT = transpose_groups(cx, pos_my, ng, f"pT{out_col}")
            augT = transpose_groups(cx, aug_full, 16, f"aT{out_col}")
            ps = cx.lp.tile([P, ng], F32, tag="ps")
            rowdot(cx, pos_my, aug_my_cols, ps, ng, f"psd{out_col}")
            lse = cx.lp.tile([P, ng], F32, tag="lse")
            for gi in range(ng):
                ttl_ps = cx.psp.tile([P, 512], F32, space="PSUM", tag="ttl",
                                     bufs=1)
                ttl = cx.lp.tile([P, BATCH], F32, tag="ttl")
                for nb_ in range(BATCH // 512):
                    nc.tensor.matmul(
                        out=ttl_ps[:, :512],
                        lhsT=posT[:D, gi * P:(gi + 1) * P],
                        rhs=augT[:D, nb_ * 512:(nb_ + 1) * 512],
                        start=True, stop=True)
                    nc.vector.tensor_copy(ttl[:, nb_ * 512:(nb_ + 1) * 512],
                                          ttl_ps[:, :512])
                mx = cx.lp.tile([P, 1], F32, tag="mx")
                nc.vector.reduce_max(mx[:],
                                     ttl[:].rearrange("p (w d) -> p w d", w=1),
                                     axis=mybir.AxisListType.X)
                nmx = cx.lp.tile([P, 1], F32, tag="nmx")
                nc.vector.tensor_scalar_mul(nmx[:], mx[:], -4.0)
                ex = cx.lp.tile([P, BATCH], F32, tag="ex")
                se = cx.lp.tile([P, 1], F32, tag="se")
                nc.scalar.activation(ex[:], ttl[:], AF.Exp, bias=nmx[:, :1],
                                     scale=4.0, accum_out=se[:, :1])
                ln = cx.lp.tile([P, 1], F32, tag="ln")
                nc.scalar.activation(ln[:], se[:], AF.Ln)
                m4 = cx.lp.tile([P, 1], F32, tag="m4")
                nc.vector.tensor_scalar_mul(m4[:], mx[:], 4.0)
                nc.vector.tensor_add(lse[:, gi:gi + 1], ln[:], m4[:])
            t4 = cx.lp.tile([P, ng], F32, tag="t4")
            nc.vector.tensor_scalar_mul(t4[:], ps[:], 4.0)
            nc.vector.tensor_tensor(out=t4[:], in0=t4[:], in1=lse[:],
                                    op=ALU.subtract)
            nc.vector.reduce_sum(part[:, out_col:out_col + 1],
                                 t4[:].rearrange("p (w d) -> p w d", w=1),
                                 axis=mybir.AxisListType.X)

        closs_partial(pos_u_il, aug_u, aug_u_my, 1)
        closs_partial(my_pos_b, aug_b, aug_b_my, 2)

        pp_ps = cx.psp.tile([P, 4], F32, space="PSUM", tag="ppps", bufs=1)
        nc.tensor.matmul(out=pp_ps[:1, :4], lhsT=ones_col[:], rhs=part[:],
                         start=True, stop=True)
        psum_sb = cx.lp.tile([1, 4], F32, tag="psums")
        nc.vector.tensor_copy(psum_sb[:], pp_ps[:1, :4])
        ar_in = cx.dramp.tile([1, 4], F32, tag="ar_in")
        ar_out = cx.dramp.tile([1, 4], F32, addr_space="Shared", tag="ar_out")
        nc.sync.dma_start(out=ar_in[:], in_=psum_sb[:])
        nc.gpsimd.collective_compute(
            "AllReduce", ALU.add, replica_groups=[list(range(NCORES))],
            ins=[ar_in[:].opt()], outs=[ar_out[:].opt()])
        fin = cx.lp.tile([1, 4], F32, tag="fin")
        nc.sync.dma_start(out=fin[:], in_=ar_out[:])
        res = cx.lp.tile([1, 2], F32, tag="res")
        nc.vector.tensor_scalar_mul(res[:, 0:1], fin[:, 0:1], 1.0 / BATCH)
        t = cx.lp.tile([1, 1], F32, tag="rt")
        nc.vector.tensor_add(t[:], fin[:, 1:2], fin[:, 2:3])
        nc.vector.tensor_scalar_mul(res[:, 1:2], t[:], -0.5 / BATCH)
        nc.sync.dma_start(out=out_t[:], in_=res[:])

        for p in reversed(es):
            p.__exit__(None, None, None)
    nc.compile()
    return nc, dbg


# ---------------------------------------------------------------- entry point

def _install_ntff_hook():
    if "antenv.axon_hooks" in sys.modules:
        return
    try:
        mod = types.ModuleType("antenv.axon_hooks")
        _hook = [None]
        mod.set_axon_ntff_profile_hook = lambda h: _hook.__setitem__(0, h)
        mod.get_axon_ntff_profile_hook = lambda: _hook[0]
        sys.modules["antenv.axon_hooks"] = mod
        import antenv
        antenv.axon_hooks = mod
        from trn_agent_boot.trn_boot import _ntff_profile_via_ctypes
        hook = _ntff_profile_via_ctypes("/opt/axon/libaxon_pjrt.so")
        if hook is not None:
            mod.set_axon_ntff_profile_hook(hook)
    except Exception:
        pass


def make_in_maps(pp):
    import ml_dtypes
    maps = []
    idx_w = {g: [wrap_idx16(pp[g]['idx'][c]) for c in range(NCORES)]
             for g in ("il", "bl", "ag")}
    lr_w = {g: stream_cols(pp[g]['lr'], ml_dtypes.bfloat16)
            for g in ("il", "bl", "ag")}
    val_w = {g: stream_cols(pp[g]['val'], ml_dtypes.bfloat16)
             for g in ("il", "bl", "ag")}
    for c in range(NCORES):
        m = {
            "f0_U": pp["f0_U"], "f0_I": pp["f0_I"], "f0_B": pp["f0_B"],
            "f0_il_sh": pp["f0_il_sh"][c], "f0_bl_sh": pp["f0_bl_sh"][c],
            "loss_aug_u": pp["aug_u"], "loss_aug_b0": pp["aug_b0"],
        }
        for g in ("il", "bl", "ag"):
            m[f"{g}_idx"] = idx_w[g][c]
            m[f"{g}_lr"] = lr_w[g][c]
            m[f"{g}_val"] = val_w[g][c]
        for k, v in pp["loss"][c].items():
            m[f"loss_{k}"] = v
        maps.append(m)
    return maps


_CACHE = {}


def kernel(**inputs) -> np.ndarray:
    _install_ntff_hook()
    pp = preprocess(inputs)
    import hashlib
    h = hashlib.md5()
    for k in ("il_row", "il_col", "bl_row", "bl_col", "agg_row", "agg_col"):
        h.update(np.asarray(inputs[k]).tobytes())
    key = h.hexdigest()
    if key not in _CACHE:
        _CACHE[key] = build(pp)
    nc, dbg = _CACHE[key]
    in_maps = make_in_maps(pp)
    trace = bool(int(os.environ.get("DSCBR_TRACE", "0")))
    res = run_bass_kernel_spmd(nc, in_maps, core_ids=list(range(NCORES)),
                               trace=trace)
    if trace and res.exec_time_ns:
        print(f"HW exec time: {res.exec_time_ns} ns")
    out = res.results[0]["out"].reshape(2).astype(np.float32)
    return out


# revision 58
# speedup vs baseline: 1.0291x; 1.0291x over previous
"""Trainium2 Bass kernel for nn_DSCBR (gnn_message_passing).

Strategy (8 NeuronCores, SPMD, dest-row sharding):
- Each graph's nodes are split into per-side regions (U/I resp. U/B),
  round-robin sharded; node tables stored bf16 padded to 256B rows so
  SWDGE dma_gather (elem_size=128 bf16) output feeds TensorE directly.
- SpMM: bank-major loop. A PSUM bank [128,512] holds 4 dest win-pairs;
  per 128-edge chunk one merged selection matrix (single is_equal vs a
  256-wide iota) drives 2 matmuls (win-pair halves) accumulating in
  PSUM across all source windows; one fused drain per bank does
  norm-epilogue + acc add + bf16 staging for the next-layer table.
- Gathers round-robin 4 SWDGE queues (descriptor-gen parallelism).
- AllGathers between layers (padded bf16) and for loss tables
  (unpadded bf16) overlap the next phase's gathers.
- Losses computed batch-sharded (256 rows/core) + tiny AllReduce.
"""
import os
import sys
import types

sys.path.insert(0, "/opt/trn_rl_repo")

import numpy as np

import concourse.bass as bass
import concourse.bacc as bacc
import concourse.mybir as mybir
import concourse.tile as tile
from concourse.bass_utils import run_bass_kernel_spmd
from concourse.masks import make_identity

P = 128
NCORES = 8
SRC_WIN = 32768
GI_CH = 20                    # chunks per gather batch (<= 2560 idx)
D = 64
NU, NI, NB = 100000, 50000, 20000
BATCH = 2048
F32 = mybir.dt.float32
BF = mybir.dt.bfloat16
I32 = mybir.dt.int32
I16 = mybir.dt.int16
AF = mybir.ActivationFunctionType
ALU = mybir.AluOpType

# per-core padded slot counts (multiples of 256)
PCU = 12544                   # 100000/8 = 12500 -> 49 wp
PCI = 6400                    # 50000/8  = 6250  -> 25 wp
PCB = 2560                    # 20000/8  = 2500  -> 10 wp
VU, VI, VB = PCU * NCORES, PCI * NCORES, PCB * NCORES   # 100352, 51200, 20480
R1 = PCU + PCI                # 18944, wps 0..73 (U 0..48, I 49..73)
R2 = PCU + PCB                # 15104, wps 0..58 (U 0..48, B 49..58)
NW1, NW2, NWB = R1 // P, R2 // P, PCB // P


def mapU(x):
    return (x % NCORES) * PCU + x // NCORES


def mapI(x):
    return (x % NCORES) * PCI + x // NCORES


def mapB(x):
    return (x % NCORES) * PCB + x // NCORES


# ---------------------------------------------------------------- host prep

def build_graph_side(dst_core, dst_wp, dst_lrow, src_idx, vals, wp0, nwp_side,
                     v_src, first_win):
    """One side of a graph: per-(core,wp,window) chunk counts (SPMD-max over
    cores) and the sorted per-core edge segments.

    Returns (banks, seg) where banks = [(bank_wp0, bank_nwp,
    [(win, [chunks...]), ...])] with chunks = (wpi, start, stop), and seg
    maps (core, wp, win) -> (slice into the sorted arrays)."""
    nwin = (v_src + SRC_WIN - 1) // SRC_WIN
    win = src_idx // SRC_WIN
    wloc = (src_idx % SRC_WIN).astype(np.int16)

    counts = np.zeros((NCORES, nwp_side, nwin), np.int64)
    np.add.at(counts, (dst_core, dst_wp - wp0, win), 1)
    nch = (counts.max(axis=0) + P - 1) // P          # [nwp_side, nwin]
    for w in range(nwp_side):
        if nch[w].sum() == 0:
            nch[w, 0] = 1

    order = np.lexsort((dst_lrow, win, dst_wp, dst_core))
    s_wp = dst_wp[order] - wp0
    s_win = win[order]
    s_core = dst_core[order]
    s_wloc = wloc[order]
    s_lrow = dst_lrow[order]
    s_val = vals[order]

    key = (s_core * nwp_side + s_wp) * nwin + s_win
    starts = np.searchsorted(key, np.arange(NCORES * nwp_side * nwin))
    ends = np.searchsorted(key, np.arange(NCORES * nwp_side * nwin) + 1)

    banks = []
    for b0 in range(0, nwp_side, 4):
        bnwp = min(4, nwp_side - b0)
        wins = []
        for w in range(nwin):
            chunks = []
            for wpi in range(bnwp):
                wp = b0 + wpi
                c = int(nch[wp, w])
                if c == 0:
                    continue
                ws = np.nonzero(nch[wp])[0]
                st_w, sp_w = ws[0], ws[-1]
                for k in range(c):
                    chunks.append((wpi, w == st_w and k == 0,
                                   w == sp_w and k == c - 1))
            if chunks:
                wins.append((w, chunks))
        banks.append((b0 + wp0, bnwp, wins))
    info = dict(banks=banks, nch=nch, starts=starts, ends=ends,
                s_wloc=s_wloc, s_lrow=s_lrow, s_val=s_val,
                nwin=nwin, wp0=wp0, nwp=nwp_side, first_win=first_win)
    return info


def fill_side_streams(info, idx_s, lr_s, val_s, pos0):
    """Append this side's padded streams for all cores in emit order.
    Returns new stream position (in edges)."""
    nwin, wp0, nwp = info['nwin'], info['wp0'], info['nwp']
    nch, starts, ends = info['nch'], info['starts'], info['ends']
    pos_out = pos0
    for c in range(NCORES):
        pos = pos0
        for (b0, bnwp, wins) in info['banks']:
            for (w, chunks) in wins:
                for wp in range(b0, b0 + bnwp):
                    ncw = int(nch[wp - wp0, w])
                    if ncw == 0:
                        continue
                    k = (c * nwp + (wp - wp0)) * nwin + w
                    a, b = starts[k], ends[k]
                    n = b - a
                    idx_s[c, pos:pos + n] = info['s_wloc'][a:b]
                    lr_s[c, pos:pos + n] = info['s_lrow'][a:b]
                    val_s[c, pos:pos + n] = info['s_val'][a:b]
                    pos += ncw * P
        pos_out = pos
    return pos_out


def wrap_idx16(flat):
    # index i -> partition i%16, col i//16; replicated x8 down partitions
    return np.ascontiguousarray(np.tile(flat.reshape(-1, 16).T.astype(np.int16), (8, 1)))


def stream_cols(a, dtype):
    # [ncores, n] -> [ncores, 128, n/128] column-chunk layout
    n = a.shape[1]
    return np.ascontiguousarray(
        a.reshape(NCORES, -1, P).transpose(0, 2, 1)).astype(dtype)


def idx_cols_i32(flat):
    n = flat.shape[0]
    assert n % P == 0
    return np.ascontiguousarray(flat.reshape(-1, P).T.astype(np.int32))


def build_graph(rows, cols, vals, kind):
    """kind: 'il' (U|I combined ids), 'bl' (U|B), 'ag' (bundle rows, item cols).
    Returns dict(sides=[sideinfo...], streams=(idx, lr, val), tot)."""
    rows = np.asarray(rows).astype(np.int64)
    cols = np.asarray(cols).astype(np.int64)
    vals = np.asarray(vals, np.float32)
    if kind == "il":
        n_first, map_dst2, side_v = NU, mapI, (VI, VU)
        nwp_a, nwp_b = PCU // 256, PCI // 256
    elif kind == "bl":
        n_first, map_dst2, side_v = NU, mapB, (VB, VU)
        nwp_a, nwp_b = PCU // 256, PCB // 256
    else:  # ag
        d = mapB(rows)
        core, slot = d // PCB, d % PCB
        s = build_graph_side(core, slot // 256, (slot % 256).astype(np.float32),
                             mapI(cols), vals, 0, PCB // 256, VI, 0)
        s['src'] = 'I1acc'
        s['v_src'] = VI
        tot = int(sum(c for (_, _, wins) in s['banks']
                      for (_, ch) in wins for c in [len(ch)])) * P
        idx_s = np.zeros((NCORES, tot), np.int16)
        lr_s = np.full((NCORES, tot), 300.0, np.float32)
        val_s = np.zeros((NCORES, tot), np.float32)
        fill_side_streams(s, idx_s, lr_s, val_s, 0)
        return dict(sides=[s], idx=idx_s, lr=lr_s, val=val_s, tot=tot)

    first = rows < n_first                      # dst in U region
    # side A: dst U, src = second region
    dA = mapU(rows[first])
    srcA = map_dst2(cols[first] - n_first)
    coreA, slotA = dA // PCU, dA % PCU
    sA = build_graph_side(coreA, slotA // 256, (slotA % 256).astype(np.float32),
                          srcA, vals[first], 0, nwp_a, side_v[0], 0)
    # side B: dst second region, src U
    d2 = map_dst2(rows[~first] - n_first)
    src2 = mapU(cols[~first])
    pc2 = PCI if kind == "il" else PCB
    core2, slot2 = d2 // pc2, d2 % pc2
    sB = build_graph_side(core2, nwp_a + slot2 // 256,
                          (slot2 % 256).astype(np.float32),
                          src2, vals[~first], nwp_a, nwp_b, side_v[1], 0)
    sA['src'] = 'I1' if kind == 'il' else 'B2'
    sA['v_src'] = side_v[0]
    sB['src'] = 'U1' if kind == 'il' else 'U2'
    sB['v_src'] = side_v[1]
    tot = 0
    for s in (sA, sB):
        tot += int(sum(len(ch) for (_, _, wins) in s['banks']
                       for (_, ch) in wins)) * P
    idx_s = np.zeros((NCORES, tot), np.int16)
    lr_s = np.full((NCORES, tot), 300.0, np.float32)
    val_s = np.zeros((NCORES, tot), np.float32)
    p = fill_side_streams(sA, idx_s, lr_s, val_s, 0)
    fill_side_streams(sB, idx_s, lr_s, val_s, p)
    return dict(sides=[sA, sB], idx=idx_s, lr=lr_s, val=val_s, tot=tot)


def preprocess(inputs):
    import ml_dtypes
    u = np.asarray(inputs["users_feature"], np.float32)
    it = np.asarray(inputs["items_feature"], np.float32)
    b = np.asarray(inputs["bundles_feature"], np.float32)

    def padded_table(feat, mapper, v):
        t = np.zeros((v, 128), ml_dtypes.bfloat16)
        t[mapper(np.arange(feat.shape[0])), :D] = feat.astype(ml_dtypes.bfloat16)
        return t

    f0_U = padded_table(u, mapU, VU)
    f0_I = padded_table(it, mapI, VI)
    f0_B = padded_table(b, mapB, VB)

    # per-core f32 shards for acc init (region layout [U | second])
    f0_il_sh = np.zeros((NCORES, R1, D), np.float32)
    f0_bl_sh = np.zeros((NCORES, R2, D), np.float32)
    for c in range(NCORES):
        nu_c = len(range(c, NU, NCORES))
        f0_il_sh[c, :nu_c] = u[c::NCORES]
        f0_bl_sh[c, :nu_c] = u[c::NCORES]
        ni_c = len(range(c, NI, NCORES))
        f0_il_sh[c, PCU:PCU + ni_c] = it[c::NCORES]
        nb_c = len(range(c, NB, NCORES))
        f0_bl_sh[c, PCU:PCU + nb_c] = b[c::NCORES]

    il = build_graph(inputs["il_row"], inputs["il_col"],
                     np.asarray(inputs["il_val"], np.float32), "il")
    bl = build_graph(inputs["bl_row"], inputs["bl_col"],
                     np.asarray(inputs["bl_val"], np.float32), "bl")
    ag = build_graph(np.asarray(inputs["agg_row"]), np.asarray(inputs["agg_col"]),
                     np.asarray(inputs["agg_val"], np.float32), "ag")

    # Laplacian factorization val = a[row]*a[col]: layer-1 staging writes
    # a-prescaled tables (t1 = a * S1/2), so layer-2 needs no per-edge val;
    # the missing a[dst] only row-scales S2, which the norm epilogue cancels.
    # AG's weight 1/deg_b is a pure dst factor applied at its drain.
    def region_col(vec_first, vec_second, pc2):
        out = np.zeros((NCORES, PCU + pc2), np.float32)
        for c in range(NCORES):
            nf = len(range(c, vec_first.shape[0], NCORES))
            out[c, :nf] = vec_first[c::NCORES]
            ns = len(range(c, vec_second.shape[0], NCORES))
            out[c, PCU:PCU + ns] = vec_second[c::NCORES]
        return out

    deg_il = np.bincount(np.asarray(inputs["il_row"]), minlength=NU + NI)
    a_il = (0.5 / (np.sqrt(deg_il) + 1e-8)).astype(np.float32)
    a_il_sh = region_col(a_il[:NU], a_il[NU:], PCI)
    deg_bl = np.bincount(np.asarray(inputs["bl_row"]), minlength=NU + NB)
    a_bl = (0.5 / (np.sqrt(deg_bl) + 1e-8)).astype(np.float32)
    a_bl_sh = region_col(a_bl[:NU], a_bl[NU:], PCB)
    deg_b = np.bincount(np.asarray(inputs["agg_row"]), minlength=NB)
    agv = (1.0 / (deg_b + 1e-8)).astype(np.float32)
    agv_sh = np.zeros((NCORES, PCB), np.float32)
    for c in range(NCORES):
        nb_c = len(range(c, NB, NCORES))
        agv_sh[c, :nb_c] = agv[c::NCORES]

    users = np.asarray(inputs["users"]).astype(np.int64)
    bundles = np.asarray(inputs["bundles"]).astype(np.int64)
    bsh = BATCH // NCORES
    loss = {}
    for c in range(NCORES):
        sl = slice(c * bsh, (c + 1) * bsh)
        loss[c] = dict(
            u_il=idx_cols_i32(mapU(users[sl])),
            b_il0=idx_cols_i32(mapB(bundles[sl, 0])),
            b_il1=idx_cols_i32(mapB(bundles[sl, 1])),
        )
    aug_u = idx_cols_i32(mapU(users))
    aug_b0 = idx_cols_i32(mapB(bundles[:, 0]))

    return dict(f0_U=f0_U, f0_I=f0_I, f0_B=f0_B,
                f0_il_sh=f0_il_sh, f0_bl_sh=f0_bl_sh,
                a_il_sh=a_il_sh, a_bl_sh=a_bl_sh, agv_sh=agv_sh,
                il=il, bl=bl, ag=ag, loss=loss,
                aug_u=aug_u, aug_b0=aug_b0)


# ---------------------------------------------------------------- bass build

class Ctx:
    pass


def emit_graph(cx, g, tables, meta, acc_sb, f1_map, scale, pos0, name,
               hook=None, fold_val=True, acol=None):
    """One SpMM layer over graph g (host program). f1_map: side-src-key ->
    (dram tile, padded:bool) for next-layer table write, or None (layer 2).
    acc_sb: [128, nw*64] f32 accumulator (norm epilogue) or None (AG copy).
    hook: called once after the first batch (to emit collectives/exports of
    the PREVIOUS phase without blocking this phase's gather dispatch)."""
    nc = cx.nc
    lr_sb, val_sb = meta
    pos = pos0
    pending_drain = [None]
    hook_pending = [hook]
    for s in g['sides']:
        table_ap = tables[s['src']]
        v_src = s['v_src']
        for (wp0, bnwp, wins) in s['banks']:
            S = 2 * bnwp
            Sl = S * D
            # one accumulation group per PSUM bank: start=True clears the
            # has_written bits of the WHOLE bank, so only the bank's first
            # matmul may set it (per-element first-touch then overwrites,
            # later touches accumulate).
            nmm = 2 * sum(len(ch) for (_, ch) in wins)
            mm = 0
            psum = cx.psp.tile([P, 512], F32, space="PSUM", tag="pbank",
                               name="pbank", bufs=3)
            for (w, chunks) in wins:
                lo = w * SRC_WIN
                rows = min(SRC_WIN, v_src - lo)
                src_slice = table_ap[lo:lo + rows, :]
                for c0 in range(0, len(chunks), GI_CH):
                    batch = chunks[c0:c0 + GI_CH]
                    nch = len(batch)
                    gi = nch * P
                    idx_t = cx.idxp.tile([128, GI_CH * 8], I16, tag="gidx",
                                         name="gidx", bufs=9)
                    nc.sync.dma_start(
                        out=idx_t[:, :gi // 16],
                        in_=cx.g_idx[name[:2]][:, pos // 16:pos // 16 + gi // 16])
                    gt = cx.gp.tile([P, GI_CH * 128], BF, tag="gg", name="gg",
                                    bufs=8)
                    nc.gpsimd.dma_gather(
                        out_ap=gt[:, :nch * 128].rearrange(
                            "p (c d) -> p c d", c=nch),
                        in_ap=src_slice,
                        idxs_ap=idx_t[:, :gi // 16],
                        num_idxs=gi,
                        num_idxs_reg=gi,
                        elem_size=128,
                        single_packet=False,
                        queue_num=cx.qrr % 4,
                    )
                    cx.qrr += 1
                    if fold_val:
                        gv = cx.gvp.tile([P, GI_CH * D], BF, tag="gv",
                                         name="gv", bufs=5)
                        nc.vector.tensor_mul(
                            gv[:, :nch * D].rearrange("p (c d) -> p c d",
                                                      c=nch),
                            gt[:, :nch * 128].rearrange(
                                "p (c d) -> p c d", c=nch)[:, :, 0:D],
                            val_sb[:, pos // P:pos // P + nch].to_broadcast(
                                [P, nch, D]))
                    else:
                        gv = gt  # rhs slices read the gather tile directly
                    sel = cx.selp.tile([P, GI_CH * 256], BF, tag="sel",
                                       name="sel", bufs=3)
                    nc.vector.tensor_tensor(
                        out=sel[:, :nch * 256].rearrange(
                            "p (c j) -> p c j", c=nch),
                        in0=cx.iota_rep[:, :nch * 256].rearrange(
                            "p (c j) -> p c j", c=nch),
                        in1=lr_sb[:, pos // P:pos // P + nch].to_broadcast(
                            [P, nch, 256]),
                        op=ALU.is_equal)
                    for k, (wpi, st, sp) in enumerate(batch):
                        if fold_val:
                            rhs = gv[:, k * D:(k + 1) * D]
                        else:
                            rhs = gt[:, k * 128:k * 128 + D]
                        nc.tensor.matmul(
                            out=psum[:, (2 * wpi) * D:(2 * wpi + 1) * D],
                            lhsT=sel[:, k * 256:k * 256 + 128],
                            rhs=rhs,
                            start=(mm == 0), stop=False)
                        nc.tensor.matmul(
                            out=psum[:, (2 * wpi + 1) * D:(2 * wpi + 2) * D],
                            lhsT=sel[:, k * 256 + 128:k * 256 + 256],
                            rhs=rhs,
                            start=False, stop=(mm == nmm - 2))
                        mm += 2
                    pos += gi
                    if hook_pending[0] is not None:
                        hook_pending[0]()
                        hook_pending[0] = None
                    # drain of the previous bank, deferred until this bank's
                    # first batch is queued (keeps it off the vector queue
                    # head while the previous bank's matmuls finish)
                    if pending_drain[0] is not None:
                        pending_drain[0]()
                        pending_drain[0] = None
            pending_drain[0] = _make_drain(cx, s, wp0, S, Sl, psum, acc_sb,
                                           f1_map, scale, acol)
    pending_drain[0]()
    return pos


def _make_drain(cx, s, wp0, S, Sl, psum, acc_sb, f1_map, scale, acol):
    nc = cx.nc

    def drain():
            w0 = wp0 * 2
            if acc_sb is not None:
                praw = cx.dp.tile([P, 512], F32, tag="d_pr", name="d_pr",
                                  bufs=2)
                nc.scalar.activation(praw[:, :Sl], psum[:, :Sl], AF.Copy)
                sq = cx.dp.tile([P, 512], F32, tag="d_sq", name="d_sq", bufs=2)
                nc.scalar.activation(sq[:, :Sl], praw[:, :Sl], AF.Square)
                ss = cx.dp.tile([P, 8], F32, tag="d_ss", name="d_ss", bufs=2)
                nc.vector.reduce_sum(
                    ss[:, :S], sq[:, :Sl].rearrange("p (s d) -> p s d", s=S),
                    axis=mybir.AxisListType.X)
                sn = cx.dp.tile([P, 8], F32, tag="d_sn", name="d_sn", bufs=2)
                nc.scalar.activation(sn[:, :S], ss[:, :S], AF.Sqrt)
                nc.vector.tensor_scalar_max(sn[:, :S], sn[:, :S], 1e-12)
                rn = cx.dp.tile([P, 8], F32, tag="d_rn", name="d_rn", bufs=2)
                nc.vector.reciprocal(rn[:, :S], sn[:, :S])
                contrib = cx.dp.tile([P, 512], F32, tag="d_ct", name="d_ct",
                                     bufs=2)
                nc.vector.tensor_mul(
                    contrib[:, :Sl].rearrange("p (s d) -> p s d", s=S),
                    praw[:, :Sl].rearrange("p (s d) -> p s d", s=S),
                    rn[:, :S].to_broadcast([P, S, D]))
                nc.vector.tensor_add(acc_sb[:, w0 * D:w0 * D + Sl],
                                     acc_sb[:, w0 * D:w0 * D + Sl],
                                     contrib[:, :Sl])
                stg_src = praw
            else:
                stg_src = None
            if f1_map is not None:
                w0 = wp0 * 2
                if stg_src is None:
                    stg_src = cx.dp.tile([P, 512], F32, tag="d_pr",
                                         name="d_pr", bufs=2)
                    nc.scalar.activation(stg_src[:, :Sl], psum[:, :Sl],
                                         AF.Copy)
                f1_t, padded = f1_map[s['src2']]
                stg = cx.stp.tile([P, 8 * D], BF, tag="d_st", name="d_st",
                                  bufs=2)
                nc.vector.tensor_mul(
                    stg[:, :Sl].rearrange("p (s d) -> p s d", s=S),
                    stg_src[:, :Sl].rearrange("p (s d) -> p s d", s=S),
                    acol[:, w0:w0 + S].to_broadcast([P, S, D]))
                r0 = (wp0 - s['wp0']) * 256
                if padded:
                    dst = f1_t[r0:r0 + S * P, 0:D]
                else:
                    dst = f1_t[r0:r0 + S * P, :]
                nc.scalar.dma_start(
                    out=dst.rearrange("(s p) d -> p s d", p=P),
                    in_=stg[:, :Sl].rearrange("p (s d) -> p s d", s=S))

    return drain


def indirect_gather_rows(cx, out_sb, table_ap, idx_sb, ncols):
    nc = cx.nc
    for k in range(ncols):
        nc.gpsimd.indirect_dma_start(
            out=out_sb[:, k * D:(k + 1) * D],
            out_offset=None,
            in_=table_ap,
            in_offset=bass.IndirectOffsetOnAxis(ap=idx_sb[:, k:k + 1], axis=0),
        )


def normalize_rows(cx, x_sb, ngroups, tag):
    nc = cx.nc
    sq = cx.lp.tile([P, ngroups * D], F32, tag=f"{tag}_sq")
    nc.vector.tensor_mul(sq[:], x_sb[:, :ngroups * D], x_sb[:, :ngroups * D])
    ss = cx.lp.tile([P, ngroups], F32, tag=f"{tag}_ss")
    nc.vector.reduce_sum(ss[:], sq[:].rearrange("p (w d) -> p w d", w=ngroups),
                         axis=mybir.AxisListType.X)
    sn = cx.lp.tile([P, ngroups], F32, tag=f"{tag}_sn")
    nc.scalar.activation(sn[:], ss[:], AF.Sqrt)
    nc.vector.tensor_scalar_max(sn[:], sn[:], 1e-12)
    rn = cx.lp.tile([P, ngroups], F32, tag=f"{tag}_rn")
    nc.vector.reciprocal(rn[:], sn[:])
    nc.vector.tensor_mul(
        x_sb[:, :ngroups * D].rearrange("p (w d) -> p w d", w=ngroups),
        x_sb[:, :ngroups * D].rearrange("p (w d) -> p w d", w=ngroups),
        rn[:].to_broadcast([P, ngroups, D]),
    )


def rowdot(cx, a_sb, b_sb, out_sb, ngroups, tag):
    nc = cx.nc
    t = cx.lp.tile([P, ngroups * D], F32, tag=f"{tag}_t")
    nc.vector.tensor_mul(t[:], a_sb[:, :ngroups * D], b_sb[:, :ngroups * D])
    nc.vector.reduce_sum(out_sb[:, :ngroups],
                         t[:].rearrange("p (w d) -> p w d", w=ngroups),
                         axis=mybir.AxisListType.X)


def transpose_groups(cx, src_sb, ngroups, tag):
    nc = cx.nc
    out = cx.lp.tile([P, ngroups * P], F32, tag=f"{tag}_T")
    for gi in range(ngroups):
        pt = cx.psp.tile([P, P], F32, space="PSUM", tag="tr_ps", bufs=1)
        nc.tensor.transpose(out=pt[:D, :P], in_=src_sb[:, gi * D:(gi + 1) * D],
                            identity=cx.ident[:])
        nc.vector.tensor_copy(out[:D, gi * P:(gi + 1) * P], pt[:D, :P])
    return out


def build(pp, debug_tables=False):
    nc = bacc.Bacc("TRN2", target_bir_lowering=False, debug=False,
                   num_devices=NCORES, num_swdge_queues=4)
    cx = Ctx()
    cx.nc = nc
    cx.qrr = 0

    # ---- dram inputs
    f0_U = nc.dram_tensor("f0_U", [VU, 128], BF, kind="ExternalInput")
    f0_I = nc.dram_tensor("f0_I", [VI, 128], BF, kind="ExternalInput")
    f0_B = nc.dram_tensor("f0_B", [VB, 128], BF, kind="ExternalInput")
    f0_il_sh = nc.dram_tensor("f0_il_sh", [R1, D], F32, kind="ExternalInput")
    f0_bl_sh = nc.dram_tensor("f0_bl_sh", [R2, D], F32, kind="ExternalInput")
    a_il_in = nc.dram_tensor("a_il_sh", [R1, 1], F32, kind="ExternalInput")
    a_bl_in = nc.dram_tensor("a_bl_sh", [R2, 1], F32, kind="ExternalInput")
    agv_in = nc.dram_tensor("agv_sh", [PCB, 1], F32, kind="ExternalInput")
    cx.g_idx = {}
    g_meta = {}
    for gname in ("il", "bl", "ag"):
        tot = pp[gname]['tot']
        cx.g_idx[gname] = nc.dram_tensor(f"{gname}_idx", [128, tot // 16], I16,
                                         kind="ExternalInput")
        g_meta[gname] = (
            nc.dram_tensor(f"{gname}_lr", [128, tot // P], BF,
                           kind="ExternalInput"),
            nc.dram_tensor(f"{gname}_val", [128, tot // P], BF,
                           kind="ExternalInput"))
    lidx = {k: nc.dram_tensor(f"loss_{k}", [128, 2], I32, kind="ExternalInput")
            for k in ("u_il", "b_il0", "b_il1")}
    lidx["aug_u"] = nc.dram_tensor("loss_aug_u", [128, 16], I32,
                                   kind="ExternalInput")
    lidx["aug_b0"] = nc.dram_tensor("loss_aug_b0", [128, 16], I32,
                                    kind="ExternalInput")
    out_t = nc.dram_tensor("out", [1, 2], F32, kind="ExternalOutput")
    dbg = {}

    # side src2 keys (which f1 tile a bank's dst side writes)
    for g, keys in (("il", ("U1f", "I1f")), ("bl", ("U2f", "B2f")),
                    ("ag", ("ilb",))):
        for s, k in zip(pp[g]['sides'], keys):
            s['src2'] = k

    with tile.TileContext(nc) as tc:
        cx.tc = tc
        es = []

        def pool(name, bufs, **kw):
            p = tc.tile_pool(name=name, bufs=bufs, **kw)
            es.append(p)
            return p.__enter__()

        cx.psp = pool("psum", 2, space="PSUM")
        cx.dramp = pool("dram", 1, space="DRAM")
        cx.cp = pool("const", 1)

        iota_i = cx.cp.tile([P, 256], I32)
        nc.gpsimd.iota(iota_i[:], pattern=[[1, 256]], base=0,
                       channel_multiplier=0)
        cx.iota_bf = cx.cp.tile([P, 256], BF)
        nc.vector.tensor_copy(cx.iota_bf[:], iota_i[:])
        cx.iota_rep = cx.cp.tile([P, GI_CH * 256], BF)
        nc.vector.tensor_copy(
            cx.iota_rep[:].rearrange("p (c j) -> p c j", c=GI_CH),
            cx.iota_bf[:].rearrange("p (o j) -> p o j", o=1).to_broadcast(
                [P, GI_CH, 256]))
        cx.ident = cx.cp.tile([P, P], F32)
        make_identity(nc, cx.ident[:])
        ones_col = cx.cp.tile([P, 1], F32)
        nc.vector.memset(ones_col[:], 1.0)

        def ag_pair(nm, rows_in, rows_out, cols, dtype=BF):
            ain = cx.dramp.tile([rows_in, cols], dtype, tag=f"{nm}_i",
                                name=f"{nm}_i")
            aout = cx.dramp.tile([rows_out, cols], dtype, addr_space="Shared",
                                 tag=f"{nm}_o", name=f"{nm}_o")
            return ain, aout

        def allgather(pair):
            nc.gpsimd.collective_compute(
                "AllGather", ALU.bypass, replica_groups=[list(range(NCORES))],
                ins=[pair[0][:].opt()], outs=[pair[1][:].opt()])

        # ---------- SpMM phases in a scoped pool block ----------
        es2 = []

        def pool2(name, bufs, **kw):
            p = tc.tile_pool(name=name, bufs=bufs, **kw)
            es2.append(p)
            return p.__enter__()

        cx.gp = pool2("gather", 4)
        cx.idxp = pool2("gidx", 6)
        cx.gvp = pool2("gval", 3)
        cx.selp = pool2("sel", 3)
        cx.dp = pool2("drain", 2)
        cx.stp = pool2("stage", 2)
        cx.mp = pool2("meta", 1)
        cx.accp = pool2("accs", 1)
        cx.xp = pool2("export", 1)

        meta_sb = {}
        for gname in ("il", "bl", "ag"):
            tot = pp[gname]['tot']
            lr = cx.mp.tile([128, tot // P], BF, tag=f"{gname}_lr",
                            name=f"{gname}_lr")
            vv = cx.mp.tile([128, tot // P], BF, tag=f"{gname}_vv",
                            name=f"{gname}_vv")
            nc.scalar.dma_start(out=lr[:], in_=g_meta[gname][0][:])
            nc.scalar.dma_start(out=vv[:], in_=g_meta[gname][1][:])
            meta_sb[gname] = (lr, vv)

        acc_il = cx.accp.tile([P, NW1 * D], F32, tag="acc_il", name="acc_il")
        nc.scalar.dma_start(out=acc_il[:].rearrange("p (w d) -> p w d", w=NW1),
                            in_=f0_il_sh[:].rearrange("(w p) d -> p w d", p=P))
        acc_bl = cx.accp.tile([P, NW2 * D], F32, tag="acc_bl", name="acc_bl")
        nc.scalar.dma_start(out=acc_bl[:].rearrange("p (w d) -> p w d", w=NW2),
                            in_=f0_bl_sh[:].rearrange("(w p) d -> p w d", p=P))
        a_il_t = cx.mp.tile([P, NW1], F32, tag="a_il", name="a_il")
        nc.scalar.dma_start(out=a_il_t[:].rearrange("p (w d) -> p w d", w=NW1),
                            in_=a_il_in[:].rearrange("(w p) d -> p w d", p=P))
        a_bl_t = cx.mp.tile([P, NW2], F32, tag="a_bl", name="a_bl")
        nc.scalar.dma_start(out=a_bl_t[:].rearrange("p (w d) -> p w d", w=NW2),
                            in_=a_bl_in[:].rearrange("(w p) d -> p w d", p=P))
        agv_t = cx.mp.tile([P, NWB], F32, tag="agv", name="agv")
        nc.scalar.dma_start(out=agv_t[:].rearrange("p (w d) -> p w d", w=NWB),
                            in_=agv_in[:].rearrange("(w p) d -> p w d", p=P))

        il_f1_U = ag_pair("ilf1U", PCU, VU, 128)
        il_f1_I = ag_pair("ilf1I", PCI, VI, 128)
        bl_f1_U = ag_pair("blf1U", PCU, VU, 128)
        bl_f1_B = ag_pair("blf1B", PCB, VB, 128)
        acc_il_I = ag_pair("accilI", PCI, VI, 128)
        acc_il_U = ag_pair("accilU", PCU, VU, D)
        acc_bl_U = ag_pair("accblU", PCU, VU, D)
        acc_bl_B = ag_pair("accblB", PCB, VB, D)
        ilb = ag_pair("ilb", PCB, VB, D)

        def export(acc_t, col0, nwin, dst, pad):
            stg = cx.xp.tile([P, (PCU // P) * D], BF, tag="xstg", name="xstg",
                             bufs=1)
            nc.scalar.activation(stg[:, :nwin * D],
                                 acc_t[:, col0 * D:(col0 + nwin) * D], AF.Copy)
            d = dst[:, 0:D] if pad else dst[:]
            nc.scalar.dma_start(
                out=d.rearrange("(s p) d -> p s d", p=P),
                in_=stg[:, :nwin * D].rearrange("p (s d) -> p s d", s=nwin))

        # ---- IL layer 1
        emit_graph(cx, pp['il'], {'I1': f0_I[:], 'U1': f0_U[:]},
                   meta_sb['il'], acc_il,
                   {'U1f': (il_f1_U[0], True), 'I1f': (il_f1_I[0], True)},
                   0.5, 0, "il1", acol=a_il_t)
        # ---- BL layer 1 (il_f1 allgathers dispatched behind its first batch)
        emit_graph(cx, pp['bl'], {'B2': f0_B[:], 'U2': f0_U[:]},
                   meta_sb['bl'], acc_bl,
                   {'U2f': (bl_f1_U[0], True), 'B2f': (bl_f1_B[0], True)},
                   0.5, 0, "bl1", acol=a_bl_t,
                   hook=lambda: (allgather(il_f1_U), allgather(il_f1_I)))
        # ---- IL layer 2 (tables are a-prescaled: no per-edge val needed)
        emit_graph(cx, pp['il'], {'I1': il_f1_I[1][:], 'U1': il_f1_U[1][:]},
                   meta_sb['il'], acc_il, None, 1.0, 0, "il2", fold_val=False,
                   hook=lambda: (allgather(bl_f1_U), allgather(bl_f1_B)))

        # ---- BL layer 2 (acc_il exports + allgathers behind first batch)
        def hook_bl2():
            export(acc_il, PCU // P, PCI // P, acc_il_I[0], True)
            export(acc_il, 0, PCU // P, acc_il_U[0], False)
            allgather(acc_il_I)
            allgather(acc_il_U)
        emit_graph(cx, pp['bl'], {'B2': bl_f1_B[1][:], 'U2': bl_f1_U[1][:]},
                   meta_sb['bl'], acc_bl, None, 1.0, 0, "bl2", hook=hook_bl2,
                   fold_val=False)

        # ---- AG spmm (copy drains into ilb)
        def hook_ag():
            export(acc_bl, 0, PCU // P, acc_bl_U[0], False)
            export(acc_bl, PCU // P, PCB // P, acc_bl_B[0], False)
            allgather(acc_bl_U)
            allgather(acc_bl_B)
        emit_graph(cx, pp['ag'], {'I1acc': acc_il_I[1][:]},
                   meta_sb['ag'], None, {'ilb': (ilb[0], False)},
                   1.0, 0, "ag", hook=hook_ag, fold_val=False, acol=agv_t)
        allgather(ilb)

        for p in reversed(es2):
            p.__exit__(None, None, None)
        cx.lp = pool("loss", 1)

        if debug_tables:
            for nm, t, rows, cols in (
                    ("dbg_acc_il_U", acc_il_U[1], VU, D),
                    ("dbg_acc_il_I", acc_il_I[1], VI, 128),
                    ("dbg_acc_bl_U", acc_bl_U[1], VU, D),
                    ("dbg_acc_bl_B", acc_bl_B[1], VB, D),
                    ("dbg_ilb", ilb[1], VB, D),
                    ("dbg_f1_U", il_f1_U[1], VU, 128),
                    ("dbg_f1_I", il_f1_I[1], VI, 128)):
                o = nc.dram_tensor(nm, [rows, cols], BF, kind="ExternalOutput")
                nc.sync.dma_start(out=o[:], in_=t[:])
                dbg[nm] = o

        # ---------------- loss phase ----------------
        bsh = BATCH // NCORES
        ng = bsh // P
        lidx_sb = {}
        for k, t in lidx.items():
            s = cx.lp.tile([128, t.shape[1]], I32, tag=f"li_{k}")
            nc.sync.dma_start(out=s[:], in_=t[:])
            lidx_sb[k] = s

        def gather(tag, table, idxk, ncols):
            sb_bf = cx.lp.tile([P, ncols * D], BF, tag=f"{tag}_bf")
            indirect_gather_rows(cx, sb_bf, table, lidx_sb[idxk], ncols)
            sb = cx.lp.tile([P, ncols * D], F32, tag=tag)
            nc.scalar.activation(sb[:], sb_bf[:], AF.Copy)
            return sb

        pos_u_il = gather("pos_u_il", acc_il_U[1][:], "u_il", ng)
        pos_u_bl = gather("pos_u_bl", acc_bl_U[1][:], "u_il", ng)
        b_il0 = gather("b_il0", ilb[1][:], "b_il0", ng)
        b_il1 = gather("b_il1", ilb[1][:], "b_il1", ng)
        b_bl0 = gather("b_bl0", acc_bl_B[1][:], "b_il0", ng)
        b_bl1 = gather("b_bl1", acc_bl_B[1][:], "b_il1", ng)
        aug_u = gather("aug_u", acc_bl_U[1][:], "aug_u", 16)
        aug_b = gather("aug_b", acc_bl_B[1][:], "aug_b0", 16)
        # -- bpr
        pr0 = cx.lp.tile([P, ng], F32, tag="pr0")
        pr1 = cx.lp.tile([P, ng], F32, tag="pr1")
        tmp = cx.lp.tile([P, ng], F32, tag="prt")
        rowdot(cx, pos_u_il, b_il0, pr0, ng, "d0")
        rowdot(cx, pos_u_bl, b_bl0, tmp, ng, "d1")
        nc.vector.tensor_add(pr0[:], pr0[:], tmp[:])
        rowdot(cx, pos_u_il, b_il1, pr1, ng, "d2")
        rowdot(cx, pos_u_bl, b_bl1, tmp, ng, "d3")
        nc.vector.tensor_add(pr1[:], pr1[:], tmp[:])
        x = cx.lp.tile([P, ng], F32, tag="bprx")
        nc.vector.tensor_tensor(out=x[:], in0=pr1[:], in1=pr0[:],
                                op=ALU.subtract)
        negx = cx.lp.tile([P, ng], F32, tag="bprnx")
        nc.vector.tensor_scalar_mul(negx[:], x[:], -1.0)
        nax = cx.lp.tile([P, ng], F32, tag="bprax")
        nc.vector.tensor_tensor(out=nax[:], in0=x[:], in1=negx[:], op=ALU.min)
        e = cx.lp.tile([P, ng], F32, tag="bpre")
        nc.scalar.activation(e[:], nax[:], AF.Exp)
        nc.vector.tensor_scalar_add(e[:], e[:], 1.0)
        l1p = cx.lp.tile([P, ng], F32, tag="bprl")
        nc.scalar.activation(l1p[:], e[:], AF.Ln)
        sp = cx.lp.tile([P, ng], F32, tag="bprsp")
        nc.vector.tensor_scalar_max(sp[:], x[:], 0.0)
        nc.vector.tensor_add(sp[:], sp[:], l1p[:])

        # -- contrastive partials
        aug_u_my = cx.lp.tile([P, ng * D], F32, tag="aug_u_my")
        nc.vector.tensor_copy(aug_u_my[:], pos_u_bl[:, :ng * D])
        aug_b_my = cx.lp.tile([P, ng * D], F32, tag="aug_b_my")
        nc.vector.tensor_copy(aug_b_my[:], b_bl0[:, :ng * D])
        my_pos_b = cx.lp.tile([P, ng * D], F32, tag="my_pb")
        nc.vector.tensor_copy(my_pos_b[:], b_il0[:, :ng * D])

        normalize_rows(cx, aug_u, 16, "nau")
        normalize_rows(cx, aug_b, 16, "nab")
        normalize_rows(cx, pos_u_il, ng, "npu")
        normalize_rows(cx, my_pos_b, ng, "npb")
        normalize_rows(cx, aug_u_my, ng, "naum")
        normalize_rows(cx, aug_b_my, ng, "nabm")

        part = cx.lp.tile([P, 4], F32, tag="parts")
        nc.vector.memset(part[:], 0.0)
        nc.vector.reduce_sum(part[:, 0:1],
                             sp[:].rearrange("p (w d) -> p w d", w=1),
                             axis=mybir.AxisListType.X)

        def closs_partial(pos_my, aug_full, aug_my_cols, out_col):
            posT = transpose_groups(cx, pos_my, ng, f"pT{out_col}")
            augT = transpose_groups(cx, aug_full, 16, f"aT{out_col}")
            ps = cx.lp.tile([P, ng], F32, tag="ps")
            rowdot(cx, pos_my, aug_my_cols, ps, ng, f"psd{out_col}")
            lse = cx.lp.tile([P, ng], F32, tag="lse")
            for gi in range(ng):
                ttl_ps = cx.psp.tile([P, 512], F32, space="PSUM", tag="ttl",
                                     bufs=1)
                ttl = cx.lp.tile([P, BATCH], F32, tag="ttl")
                for nb_ in range(BATCH // 512):
                    nc.tensor.matmul(
                        out=ttl_ps[:, :512],
                        lhsT=posT[:D, gi * P:(gi + 1) * P],
                        rhs=augT[:D, nb_ * 512:(nb_ + 1) * 512],
                        start=True, stop=True)
                    nc.vector.tensor_copy(ttl[:, nb_ * 512:(nb_ + 1) * 512],
                                          ttl_ps[:, :512])
                mx = cx.lp.tile([P, 1], F32, tag="mx")
                nc.vector.reduce_max(mx[:],
                                     ttl[:].rearrange("p (w d) -> p w d", w=1),
                                     axis=mybir.AxisListType.X)
                nmx = cx.lp.tile([P, 1], F32, tag="nmx")
                nc.vector.tensor_scalar_mul(nmx[:], mx[:], -4.0)
                ex = cx.lp.tile([P, BATCH], F32, tag="ex")
                se = cx.lp.tile([P, 1], F32, tag="se")
                nc.scalar.activation(ex[:], ttl[:], AF.Exp, bias=nmx[:, :1],
                                     scale=4.0, accum_out=se[:, :1])
                ln = cx.lp.tile([P, 1], F32, tag="ln")
                nc.scalar.activation(ln[:], se[:], AF.Ln)
                m4 = cx.lp.tile([P, 1], F32, tag="m4")
                nc.vector.tensor_scalar_mul(m4[:], mx[:], 4.0)
                nc.vector.tensor_add(lse[:, gi:gi + 1], ln[:], m4[:])
            t4 = cx.lp.tile([P, ng], F32, tag="t4")
            nc.vector.tensor_scalar_mul(t4[:], ps[:], 4.0)
            nc.vector.tensor_tensor(out=t4[:], in0=t4[:], in1=lse[:],
                                    op=ALU.subtract)
            nc.vector.reduce_sum(part[:, out_col:out_col + 1],
                                 t4[:].rearrange("p (w d) -> p w d", w=1),
                                 axis=mybir.AxisListType.X)

        closs_partial(pos_u_il, aug_u, aug_u_my, 1)
        closs_partial(my_pos_b, aug_b, aug_b_my, 2)

        pp_ps = cx.psp.tile([P, 4], F32, space="PSUM", tag="ppps", bufs=1)
        nc.tensor.matmul(out=pp_ps[:1, :4], lhsT=ones_col[:], rhs=part[:],
                         start=True, stop=True)
        psum_sb = cx.lp.tile([1, 4], F32, tag="psums")
        nc.vector.tensor_copy(psum_sb[:], pp_ps[:1, :4])
        ar_in = cx.dramp.tile([1, 4], F32, tag="ar_in")
        ar_out = cx.dramp.tile([1, 4], F32, addr_space="Shared", tag="ar_out")
        nc.sync.dma_start(out=ar_in[:], in_=psum_sb[:])
        nc.gpsimd.collective_compute(
            "AllReduce", ALU.add, replica_groups=[list(range(NCORES))],
            ins=[ar_in[:].opt()], outs=[ar_out[:].opt()])
        fin = cx.lp.tile([1, 4], F32, tag="fin")
        nc.sync.dma_start(out=fin[:], in_=ar_out[:])
        res = cx.lp.tile([1, 2], F32, tag="res")
        nc.vector.tensor_scalar_mul(res[:, 0:1], fin[:, 0:1], 1.0 / BATCH)
        t = cx.lp.tile([1, 1], F32, tag="rt")
        nc.vector.tensor_add(t[:], fin[:, 1:2], fin[:, 2:3])
        nc.vector.tensor_scalar_mul(res[:, 1:2], t[:], -0.5 / BATCH)
        nc.sync.dma_start(out=out_t[:], in_=res[:])

        for p in reversed(es):
            p.__exit__(None, None, None)
    nc.compile()
    return nc, dbg


# ---------------------------------------------------------------- entry point

def _install_ntff_hook():
    if "antenv.axon_hooks" in sys.modules:
        return
    try:
        mod = types.ModuleType("antenv.axon_hooks")
        _hook = [None]
        mod.set_axon_ntff_profile_hook = lambda h: _hook.__setitem__(0, h)
        mod.get_axon_ntff_profile_hook = lambda: _hook[0]
        sys.modules["antenv.axon_hooks"] = mod
        import antenv
        antenv.axon_hooks = mod
        from trn_agent_boot.trn_boot import _ntff_profile_via_ctypes
        hook = _ntff_profile_via_ctypes("/opt/axon/libaxon_pjrt.so")
        if hook is not None:
            mod.set_axon_ntff_profile_hook(hook)
    except Exception:
        pass


def make_in_maps(pp):
    import ml_dtypes
    maps = []
    idx_w = {g: [wrap_idx16(pp[g]['idx'][c]) for c in range(NCORES)]
             for g in ("il", "bl", "ag")}
    lr_w = {g: stream_cols(pp[g]['lr'], ml_dtypes.bfloat16)
            for g in ("il", "bl", "ag")}
    val_w = {g: stream_cols(pp[g]['val'], ml_dtypes.bfloat16)
             for g in ("il", "bl", "ag")}
    for c in range(NCORES):
        m = {
            "f0_U": pp["f0_U"], "f0_I": pp["f0_I"], "f0_B": pp["f0_B"],
            "f0_il_sh": pp["f0_il_sh"][c], "f0_bl_sh": pp["f0_bl_sh"][c],
            "a_il_sh": pp["a_il_sh"][c][:, None],
            "a_bl_sh": pp["a_bl_sh"][c][:, None],
            "agv_sh": pp["agv_sh"][c][:, None],
            "loss_aug_u": pp["aug_u"], "loss_aug_b0": pp["aug_b0"],
        }
        for g in ("il", "bl", "ag"):
            m[f"{g}_idx"] = idx_w[g][c]
            m[f"{g}_lr"] = lr_w[g][c]
            m[f"{g}_val"] = val_w[g][c]
        for k, v in pp["loss"][c].items():
            m[f"loss_{k}"] = v
        maps.append(m)
    return maps


_CACHE = {}


def kernel(**inputs) -> np.ndarray:
    _install_ntff_hook()
    pp = preprocess(inputs)
    import hashlib
    h = hashlib.md5()
    for k in ("il_row", "il_col", "bl_row", "bl_col", "agg_row", "agg_col"):
        h.update(np.asarray(inputs[k]).tobytes())
    key = h.hexdigest()
    if key not in _CACHE:
        _CACHE[key] = build(pp)
    nc, dbg = _CACHE[key]
    in_maps = make_in_maps(pp)
    trace = bool(int(os.environ.get("DSCBR_TRACE", "0")))
    res = run_bass_kernel_spmd(nc, in_maps, core_ids=list(range(NCORES)),
                               trace=trace)
    if trace and res.exec_time_ns:
        print(f"HW exec time: {res.exec_time_ns} ns")
    out = res.results[0]["out"].reshape(2).astype(np.float32)
    return out
